# revision 32
# baseline (speedup 1.0000x reference)
"""EmergentSpinGlass fused kernel for 8 Trainium2 NeuronCores.

Reference computation (per batch b):
    s   = x @ W_spin.T + b_spin                       (N, D)
    mf  = mean_n s                                    (D,)
    g   = W_global @ mf                               (D,)   [same for all rows]
    EF  = s @ W_J.T                                   (N, D)
    A   = softmax(EF @ s.T / sqrt(D), axis=-1)        (N, N)
    LF  = A @ s                                       (N, D)
    out = tanh(beta * (s + g + LF))                   (N, D)

Sharding: 8 cores = 4 batches x 2 query-halves. Each core receives x^T for
its batch with its query half's rows permuted first (attention is
permutation-invariant over keys), computes s for all 2048 keys, and runs
the attention block for its 1024 queries. Weights are pre-transposed on
the host; all device matmuls contract over the SBUF partition dim.

Precision plan: s for the OWN query half (keys 0..1023 after the
permutation) runs in bf16 — it feeds the dominant s-term of the output.
s for the PARTNER half (keys 1024..2047) only feeds the attention path
(scores columns, SN rows of the local field, mean-field), which is fp8
anyway, so it is computed directly with fp8 DoubleRow matmuls at 2x PE
throughput. EF, scores, and the local field all run in fp8e4m3
DoubleRow. Softmax is stored UNNORMALIZED (exp of the scaled scores is
~e^+-1, ideally ranged for e4m3; normalized values ~1/2048 would hit fp8
subnormals), and the 1/Z normalization is applied to the 512-wide LF
result instead of the 2048-wide P. The constant g term is pre-broadcast
once and folded into the s-term tile (SQG = s + g) during the transpose
copies.

Softmax skips the running-max subtraction: scaled scores for this
problem's distribution are bounded (|scores|/sqrt(D) < ~2 with huge
margin), so exp() cannot overflow; softmax itself is shift-invariant.

Structure, tuned from hardware profiles:
  - x^T is streamed chunk-major (contiguous per 512-key chunk) with the
    first chunk split by k-tile so the first matmul starts as soon as
    ~0.4MB has landed (DMA queues open at ~9us into the NEFF).
  - ST8 (fp8 copy of s^T, the moving operand of EF/scores) is produced
    by the scalar engine during phase 1, where it is otherwise idle.
  - EF runs between key chunks 2 and 3 of phase 1 (it only needs the
    query chunks 0,1), filling the ST8-conversion latency bubble.
  - the SN transposes are split around the g computation: partner-half
    tiles right after chunk 3 (covering the mean-field DVE drain), then
    the tiny g matmuls, then own-half tiles whose SQG = s + g copies
    need G. scores(0,1) run last so the ST8 scalar-engine conversions
    have drained before the attention pipeline starts.
  - phase 5 is software-pipelined: scores/exp of query tile i overlap the
    P-transpose + local-field matmuls of tile i-2; scores PSUM is a ring
    of single-bank tiles so scores(i) never waits on exp(i-1) draining.
  - PE transposes write 4 tiles into one PSUM bank before a single
    512-wide copy (copy cost is latency-dominated).
"""

import numpy as np
import ml_dtypes

import concourse.bass as bass
import concourse.tile as tile
from concourse import bacc, mybir
from concourse import bass_utils
from concourse.masks import make_identity
from concourse.bass_interp import get_hw_module

F32 = mybir.dt.float32
BF16 = mybir.dt.bfloat16
FP8 = mybir.dt.float8e4
ADD = mybir.AluOpType.add
MULT = mybir.AluOpType.mult
DR = mybir.MatmulPerfMode.DoubleRow

B, N, D = 4, 2048, 1024
NQ = N // 2          # queries per core
KT = D // 128        # 8 contraction tiles
MT = N // 128        # 16 key tiles
QT = NQ // 128       # 8 query tiles
NCH = N // 512       # 4 key chunks of 512
SCALE = 1.0 / np.sqrt(np.float32(D))

MM_MODE = "fp8"

LAST_RESULT = None   # BassKernelResults of the most recent run (for test.py)
_CACHED = {}


def _build_fp8(debug=False):
    nc = bacc.Bacc(
        "TRN2",
        target_bir_lowering=False,
        debug=False,
        enable_asserts=False,
        num_devices=8,
    )
    # x^T chunk-major: [128, chunk, kt, 512]; bf16 for own-half chunks 0,1
    # and fp8 for partner-half chunks 2,3
    xt_d = nc.dram_tensor("xt", [128, 2, KT, 512], BF16,
                          kind="ExternalInput").ap()
    xt8_d = nc.dram_tensor("xt8", [128, 2, KT, 512], FP8,
                           kind="ExternalInput").ap()
    wspin_d = nc.dram_tensor("wspinT", [128, KT, D], BF16,
                             kind="ExternalInput").ap()
    wspin8_d = nc.dram_tensor("wspinT8", [128, KT, D], FP8,
                              kind="ExternalInput").ap()
    wj_d = nc.dram_tensor("wjT", [128, KT, D], FP8, kind="ExternalInput").ap()
    wglob_d = nc.dram_tensor("wglobT", [128, KT, D], BF16,
                             kind="ExternalInput").ap()
    bspin_d = nc.dram_tensor("bspin", [128, KT], F32, kind="ExternalInput").ap()
    beta_d = nc.dram_tensor("beta", [1, 1], F32, kind="ExternalInput").ap()
    out_d = nc.dram_tensor("out", [NQ, D], F32, kind="ExternalOutput").ap()

    with tile.TileContext(nc) as tc:
        with (
            tc.tile_pool(name="const", bufs=1) as const,
            tc.tile_pool(name="longp", bufs=1) as longp,
            tc.tile_pool(name="stats", bufs=8) as stats,
            tc.tile_pool(name="rinvp", bufs=4) as rinvp,
        ):
            ident32 = const.tile([128, 128], F32)
            make_identity(nc, ident32)
            ident_a = const.tile([128, 128], BF16)
            nc.vector.tensor_copy(ident_a[:], ident32[:])
            ones1 = const.tile([1, 128], BF16)
            nc.vector.memset(ones1, 1.0)
            beta_sb = const.tile([128, 1], F32)
            nc.gpsimd.dma_start(out=beta_sb[:], in_=beta_d.to_broadcast((128, 1)))
            bspin_sb = const.tile([128, KT], F32)
            nc.gpsimd.dma_start(out=bspin_sb[:], in_=bspin_d[:])
            mf4 = const.tile([128, KT, NCH], F32)
            mf = const.tile([128, KT], F32)
            mfs = const.tile([128, KT], BF16)
            gT = const.tile([1, D], BF16)
            G_sb = const.tile([128, D], F32)   # g broadcast to all partitions

            ST = longp.tile([128, KT, N], BF16)    # s^T: [d-in-tile, d-tile, key]
            ST8 = longp.tile([128, KT, N], FP8)    # fp8 copy for EF/scores moving
            SN = longp.tile([128, MT, D], FP8)     # [key-in-tile, key-tile, d]
            SQG = longp.tile([128, QT, D], F32)    # s + g for own queries

            # EF8 outlives phase 1; enter before ph1 (LIFO close order)
            efp_cm = tc.tile_pool(name="efp", bufs=1)
            efp = efp_cm.__enter__()
            EF8 = efp.tile([128, KT, NQ], FP8)  # [d-in-tile, d-tile, query]

            # ---- Phase 1 (+EF +transposes +g) ----
            with tc.tile_pool(name="ph1", bufs=1) as ph1:
                wspin_sb = ph1.tile([128, KT, D], BF16)
                wspin8_sb = ph1.tile([128, KT, D], FP8)
                wj8 = ph1.tile([128, KT, D], FP8)
                wglob_sb = ph1.tile([128, KT, D], BF16)
                xtc = {}

                def load_chunk(nch, kta=0, ktb=KT):
                    if nch not in xtc:
                        dt = BF16 if nch < 2 else FP8
                        xtc[nch] = ph1.tile([128, KT, 512], dt,
                                            name=f"xtc{nch}", tag=f"xtc{nch % 2}",
                                            bufs=1)
                    src = xt_d if nch < 2 else xt8_d
                    nc.sync.dma_start(
                        out=xtc[nch][:, kta:ktb, :],
                        in_=src[:, nch % 2, kta:ktb, :])

                # interleave weight/x loads so the first 8 matmuls are gated
                # on only ~0.4MB of DMA, and each later pass's data arrives
                # just ahead of the PE
                nc.sync.dma_start(out=wspin_sb[:, 0:1, :], in_=wspin_d[:, 0:1, :])
                load_chunk(0, 0, 1)
                nc.sync.dma_start(out=wspin_sb[:, 1:2, :], in_=wspin_d[:, 1:2, :])
                load_chunk(0, 1, 2)
                nc.sync.dma_start(out=wspin_sb[:, 2:4, :], in_=wspin_d[:, 2:4, :])
                load_chunk(0, 2, 4)
                nc.sync.dma_start(out=wspin_sb[:, 4:8, :], in_=wspin_d[:, 4:8, :])
                load_chunk(0, 4, 8)
                load_chunk(1)

                # chunk 0 in kt-split passes so matmuls start early
                with tc.tile_pool(name="ps1a", bufs=1, space="PSUM") as ps1a:
                    ps_n0 = [ps1a.tile([128, 512], F32, name=f"psn0_{ot}",
                                       tag=f"psn0_{ot}")
                             for ot in range(KT)]
                    kt0 = 0
                    for pi, klen in enumerate((1, 1, 2, 4)):
                        for ot in range(KT):
                            for kt in range(kt0, kt0 + klen):
                                nc.tensor.matmul(
                                    ps_n0[ot][:],
                                    wspin_sb[:, kt, ot * 128:(ot + 1) * 128],
                                    xtc[0][:, kt, :],
                                    start=(kt == 0), stop=(kt == KT - 1),
                                )
                        kt0 += klen
                        if pi == 0:
                            # queue the remaining input DMA behind the hot ones
                            nc.sync.dma_start(out=wj8[:], in_=wj_d[:])
                            nc.sync.dma_start(out=wspin8_sb[:], in_=wspin8_d[:])
                            load_chunk(2)
                            load_chunk(3)
                            nc.sync.dma_start(out=wglob_sb[:], in_=wglob_d[:])
                    for ot in range(KT):
                        nc.vector.tensor_scalar(
                            out=ST[:, ot, 0:512],
                            in0=ps_n0[ot][:],
                            scalar1=bspin_sb[:, ot:ot + 1],
                            scalar2=None,
                            op0=ADD, op1=ADD,
                            accum_out=mf4[:, ot, 0:1],
                        )
                        nc.scalar.copy(ST8[:, ot, 0:512], ST[:, ot, 0:512])

                with (
                    tc.tile_pool(name="ps1", bufs=3, space="PSUM") as ps1,
                    tc.tile_pool(name="ps2", bufs=1, space="PSUM") as ps2,
                    tc.tile_pool(name="ps3", bufs=3, space="PSUM") as ps3,
                ):
                    def s_epilogue(ps, ot, nch):
                        sl = slice(nch * 512, (nch + 1) * 512)
                        nc.vector.tensor_scalar(
                            out=ST[:, ot, sl],
                            in0=ps[:],
                            scalar1=bspin_sb[:, ot:ot + 1],
                            scalar2=None,
                            op0=ADD, op1=ADD,
                            accum_out=mf4[:, ot, nch:nch + 1],
                        )
                        nc.scalar.copy(ST8[:, ot, sl], ST[:, ot, sl])

                    # chunk 1: bf16 (own-half keys = own queries)
                    for ot in range(KT):
                        ps = ps1.tile([128, 512], F32)
                        for kt in range(KT):
                            nc.tensor.matmul(
                                ps[:],
                                wspin_sb[:, kt, ot * 128:(ot + 1) * 128],
                                xtc[1][:, kt, :],
                                start=(kt == 0), stop=(kt == KT - 1),
                            )
                        s_epilogue(ps, ot, 1)

                    # chunk 2: fp8 DoubleRow (partner half, attention-only)
                    for ot in range(KT):
                        ps = ps1.tile([128, 512], F32)
                        for j in range(KT // 2):
                            nc.tensor.matmul(
                                ps[:],
                                wspin8_sb[:, 2 * j:2 * j + 2,
                                          ot * 128:(ot + 1) * 128],
                                xtc[2][:, 2 * j:2 * j + 2, :],
                                start=(j == 0), stop=(j == KT // 2 - 1),
                                perf_mode=DR,
                            )
                        s_epilogue(ps, ot, 2)

                    # EF8 = W_J^T . s^T[queries] (needs only chunks 0,1)
                    for ot in range(KT):
                        for ch in range(2):
                            ps = ps1.tile([128, 512], F32)
                            for j in range(KT // 2):
                                nc.tensor.matmul(
                                    ps[:],
                                    wj8[:, 2 * j:2 * j + 2,
                                        ot * 128:(ot + 1) * 128],
                                    ST8[:, 2 * j:2 * j + 2,
                                        ch * 512:(ch + 1) * 512],
                                    start=(j == 0), stop=(j == KT // 2 - 1),
                                    perf_mode=DR,
                                )
                            nc.vector.tensor_copy(
                                EF8[:, ot, ch * 512:(ch + 1) * 512], ps[:]
                            )

                    # chunk 3: fp8 DoubleRow
                    for ot in range(KT):
                        ps = ps1.tile([128, 512], F32)
                        for j in range(KT // 2):
                            nc.tensor.matmul(
                                ps[:],
                                wspin8_sb[:, 2 * j:2 * j + 2,
                                          ot * 128:(ot + 1) * 128],
                                xtc[3][:, 2 * j:2 * j + 2, :],
                                start=(j == 0), stop=(j == KT // 2 - 1),
                                perf_mode=DR,
                            )
                        s_epilogue(ps, ot, 3)

                    # mean field (DVE; drains behind chunk-3 epilogues)
                    for ot in range(KT):
                        nc.vector.reduce_sum(
                            out=mf[:, ot:ot + 1], in_=mf4[:, ot, :],
                            axis=mybir.AxisListType.X,
                        )
                    nc.vector.tensor_scalar_mul(mfs[:], mf[:], 1.0 / N)

                    def transpose_block(mt):
                        for dq in range(KT // 4):
                            tp = ps3.tile([128, 4, 128], BF16, name="tp",
                                          tag="tp", bufs=3)
                            for j in range(4):
                                nc.tensor.transpose(
                                    tp[:, j, :],
                                    ST[:, dq * 4 + j, mt * 128:(mt + 1) * 128],
                                    ident_a[:],
                                )
                            dsl4 = slice(dq * 512, (dq + 1) * 512)
                            # split fp8 copies across ACT/DVE
                            if dq % 2 == 0:
                                nc.scalar.copy(SN[:, mt, dsl4], tp[:])
                            else:
                                nc.vector.tensor_copy(SN[:, mt, dsl4], tp[:])
                            if mt < QT:
                                # fold the constant g in: SQG = s + g
                                nc.vector.tensor_tensor(
                                    out=SQG[:, mt, dsl4],
                                    in0=tp[:].rearrange("p a b -> p (a b)"),
                                    in1=G_sb[:, dsl4],
                                    op=ADD,
                                )

                    # partner-half transposes (no SQG -> no dependence on g);
                    # they cover the mean-field DVE drain
                    for mt in range(QT, MT):
                        transpose_block(mt)

                    # g^T = mf^T . W_global^T, then broadcast to partitions
                    gps = ps2.tile([1, 2, 512], F32)
                    for ch in range(2):
                        for dt_ in range(KT):
                            nc.tensor.matmul(
                                gps[:, ch, :],
                                mfs[:, dt_:dt_ + 1],
                                wglob_sb[:, dt_, ch * 512:(ch + 1) * 512],
                                start=(dt_ == 0), stop=(dt_ == KT - 1),
                            )
                    nc.vector.tensor_copy(
                        gT[:], gps[0:1, :, :].rearrange("p a b -> p (a b)")
                    )
                    for ch in range(2):
                        ps = ps1.tile([128, 512], F32)
                        nc.tensor.matmul(
                            ps[:], ones1[:], gT[:, ch * 512:(ch + 1) * 512],
                            start=True, stop=True)
                        nc.scalar.copy(G_sb[:, ch * 512:(ch + 1) * 512], ps[:])

                    # own-half transposes (SQG needs G ready)
                    for mt in range(QT):
                        transpose_block(mt)

            # ---- Attention: scores(0,1), then the pipelined
            # scores/softmax + P-transpose/local-field loop.
            with (
                tc.tile_pool(name="work", bufs=2) as work,
                tc.tile_pool(name="ps5s", bufs=4, space="PSUM") as ps5s,
            ):
                def scores_softmax(qt):
                    q0 = qt * 128
                    P_sb = work.tile([128, N], BF16, bufs=3)
                    rs4 = stats.tile([128, NCH], F32)
                    for mch in range(NCH):
                        ps_s = ps5s.tile([128, 512], F32)
                        for j in range(KT // 2):
                            nc.tensor.matmul(
                                ps_s[:],
                                EF8[:, 2 * j:2 * j + 2, q0:q0 + 128],
                                ST8[:, 2 * j:2 * j + 2,
                                    mch * 512:(mch + 1) * 512],
                                start=(j == 0), stop=(j == KT // 2 - 1),
                                perf_mode=DR,
                            )
                        # no max subtraction: |scores|*SCALE < ~2 here.
                        # P stays UNNORMALIZED (fp8-friendly range).
                        nc.scalar.activation(
                            out=P_sb[:, mch * 512:(mch + 1) * 512],
                            in_=ps_s[:],
                            func=mybir.ActivationFunctionType.Exp,
                            bias=0.0, scale=float(SCALE),
                            accum_out=rs4[:, mch:mch + 1],
                        )
                    rs = stats.tile([128, 1], F32)
                    nc.vector.reduce_sum(out=rs[:], in_=rs4[:],
                                         axis=mybir.AxisListType.X)
                    rinv = rinvp.tile([128, 1], F32)
                    nc.vector.reciprocal(rinv[:], rs[:])
                    return P_sb, rinv

                live = {}
                live[0] = scores_softmax(0)
                live[1] = scores_softmax(1)

                ps5t_cm = tc.tile_pool(name="ps5t", bufs=2, space="PSUM")
                ps5t = ps5t_cm.__enter__()
                ps5l_cm = tc.tile_pool(name="ps5l", bufs=2, space="PSUM")
                ps5l = ps5l_cm.__enter__()

                def pt_lf(qt, P_rinv):
                    P_sb, rinv = P_rinv
                    q0 = qt * 128
                    PT = work.tile([128, MT, 128], FP8)
                    for mq in range(MT // 4):
                        tp2 = ps5t.tile([128, 4, 128], BF16)
                        for j in range(4):
                            mt = mq * 4 + j
                            nc.tensor.transpose(
                                tp2[:, j, :],
                                P_sb[:, mt * 128:(mt + 1) * 128],
                                ident_a[:],
                            )
                        nc.vector.tensor_copy(
                            PT[:, mq * 4:(mq + 1) * 4, :], tp2[:])
                    for dch in range(2):
                        dsl = slice(dch * 512, (dch + 1) * 512)
                        plf = ps5l.tile([128, 512], F32)
                        for j in range(MT // 2):
                            nc.tensor.matmul(
                                plf[:],
                                PT[:, 2 * j:2 * j + 2, :],
                                SN[:, 2 * j:2 * j + 2, dsl],
                                start=(j == 0), stop=(j == MT // 2 - 1),
                                perf_mode=DR,
                            )
                        # z = LF/Z + (s + g); out = tanh(beta * z).
                        # The very last tile runs in 256-wide slices so the
                        # final DVE->ACT->DMA chain after the last matmul is
                        # half as long.
                        nsl = 2 if (qt == QT - 1 and dch == 1) else 1
                        w = 512 // nsl
                        for si in range(nsl):
                            ssl = slice(si * w, (si + 1) * w)
                            osl = slice(dch * 512 + si * w,
                                        dch * 512 + (si + 1) * w)
                            z = work.tile([128, 512], F32, name="z", tag="z",
                                          bufs=2)
                            nc.vector.scalar_tensor_tensor(
                                out=z[:, ssl], in0=plf[:, ssl], scalar=rinv[:],
                                in1=SQG[:, qt, osl], op0=MULT, op1=ADD,
                            )
                            osb = work.tile([128, 512], F32, name="osb",
                                            tag="osb", bufs=4)
                            nc.scalar.activation(
                                out=osb[:, ssl], in_=z[:, ssl],
                                func=mybir.ActivationFunctionType.Tanh,
                                bias=0.0, scale=beta_sb[:],
                            )
                            nc.sync.dma_start(
                                out=out_d[q0:q0 + 128, osl], in_=osb[:, ssl])

                for i in range(2, QT + 2):
                    if i < QT:
                        live[i] = scores_softmax(i)
                    pt_lf(i - 2, live.pop(i - 2))
                ps5l_cm.__exit__(None, None, None)
                ps5t_cm.__exit__(None, None, None)

            efp_cm.__exit__(None, None, None)

    nc.compile()
    nc.m = get_hw_module(nc.m)
    return nc


def _tile_kxm(a, np_dt):
    """(K, M) row-major -> [128, K//128, M] with k = kt*128 + p."""
    k, m = a.shape
    return np.ascontiguousarray(
        a.reshape(k // 128, 128, m).transpose(1, 0, 2)
    ).astype(np_dt)


def kernel(x, W_spin, b_spin, W_global, W_J, beta):
    global LAST_RESULT
    x = np.asarray(x, dtype=np.float32)
    W_spin = np.asarray(W_spin, dtype=np.float32)
    b_spin = np.asarray(b_spin, dtype=np.float32)
    W_global = np.asarray(W_global, dtype=np.float32)
    W_J = np.asarray(W_J, dtype=np.float32)
    beta = np.asarray(beta, dtype=np.float32)

    if MM_MODE not in _CACHED:
        _CACHED[MM_MODE] = _build_fp8()
    nc = _CACHED[MM_MODE]

    wspinT = _tile_kxm(W_spin.T, ml_dtypes.bfloat16)   # W_spin.T is (k, o)
    wspinT8 = _tile_kxm(W_spin.T, ml_dtypes.float8_e4m3)
    wjT = _tile_kxm(W_J.T, ml_dtypes.float8_e4m3)
    wglobT = _tile_kxm(W_global.T, ml_dtypes.bfloat16)
    bspin = np.ascontiguousarray(b_spin.reshape(KT, 128).T).astype(np.float32)
    beta_h = beta.reshape(1, 1).astype(np.float32)

    in_maps = []
    for core in range(8):
        b, h = divmod(core, 2)
        xb = x[b]
        if h == 0:
            x_perm = xb
        else:
            x_perm = np.concatenate([xb[NQ:], xb[:NQ]], axis=0)
        xt_full = _tile_kxm(np.ascontiguousarray(x_perm.T), np.float32)
        # [128, KT, N] -> chunk-major [128, NCH, KT, 512]
        xt_full = xt_full.reshape(128, KT, NCH, 512).transpose(0, 2, 1, 3)
        xt = np.ascontiguousarray(xt_full[:, 0:2]).astype(ml_dtypes.bfloat16)
        xt8 = np.ascontiguousarray(xt_full[:, 2:4]).astype(
            ml_dtypes.float8_e4m3)
        in_maps.append({
            "xt": xt, "xt8": xt8, "wspinT": wspinT, "wspinT8": wspinT8,
            "wjT": wjT, "wglobT": wglobT, "bspin": bspin, "beta": beta_h,
        })

    LAST_RESULT = bass_utils.run_bass_kernel_spmd(
        nc, in_maps, core_ids=list(range(8))
    )

    out = np.empty((B, N, D), dtype=np.float32)
    for core in range(8):
        b, h = divmod(core, 2)
        out[b, h * NQ:(h + 1) * NQ, :] = LAST_RESULT.results[core]["out"]
    return out


# revision 33
# speedup vs baseline: 1.0187x; 1.0187x over previous
"""EmergentSpinGlass fused kernel for 8 Trainium2 NeuronCores.

Reference computation (per batch b):
    s   = x @ W_spin.T + b_spin                       (N, D)
    mf  = mean_n s                                    (D,)
    g   = W_global @ mf                               (D,)   [same for all rows]
    EF  = s @ W_J.T                                   (N, D)
    A   = softmax(EF @ s.T / sqrt(D), axis=-1)        (N, N)
    LF  = A @ s                                       (N, D)
    out = tanh(beta * (s + g + LF))                   (N, D)

Sharding: 8 cores = 4 batches x 2 query-halves. Each core receives x^T for
its batch with its query half's rows permuted first (attention is
permutation-invariant over keys), computes s for all 2048 keys, and runs
the attention block for its 1024 queries. Weights are pre-transposed on
the host; all device matmuls contract over the SBUF partition dim.

Precision plan: s for the OWN query half (keys 0..1023 after the
permutation) runs in bf16 — it feeds the dominant s-term of the output.
s for the PARTNER half (keys 1024..2047) only feeds the attention path
(scores columns, SN rows of the local field, mean-field), which is fp8
anyway, so it is computed directly with fp8 DoubleRow matmuls at 2x PE
throughput. EF, scores, and the local field all run in fp8e4m3
DoubleRow. Softmax is stored UNNORMALIZED (exp of the scaled scores is
~e^+-1, ideally ranged for e4m3; normalized values ~1/2048 would hit fp8
subnormals), and the 1/Z normalization is applied to the 512-wide LF
result instead of the 2048-wide P. The constant g term is pre-broadcast
once and folded into the s-term tile (SQG = s + g) during the transpose
copies.

Softmax skips the running-max subtraction: scaled scores for this
problem's distribution are bounded (|scores|/sqrt(D) < ~2 with huge
margin), so exp() cannot overflow; softmax itself is shift-invariant.

Structure, tuned from hardware profiles:
  - x^T is streamed chunk-major (contiguous per 512-key chunk) with the
    first chunk split by k-tile so the first matmul starts as soon as
    ~0.4MB has landed (DMA queues open at ~9us into the NEFF).
  - ST8 (fp8 copy of s^T, the moving operand of EF/scores) is produced
    by the scalar engine during phase 1, where it is otherwise idle.
  - EF runs between key chunks 2 and 3 of phase 1 (it only needs the
    query chunks 0,1), filling the ST8-conversion latency bubble.
  - the SN transposes are split around the g computation: partner-half
    tiles right after chunk 3 (covering the mean-field DVE drain), then
    the tiny g matmuls, then own-half tiles whose SQG = s + g copies
    need G. scores(0,1) run last so the ST8 scalar-engine conversions
    have drained before the attention pipeline starts.
  - phase 5 is software-pipelined: scores/exp of query tile i overlap the
    P-transpose + local-field matmuls of tile i-2; scores PSUM is a ring
    of single-bank tiles so scores(i) never waits on exp(i-1) draining.
  - PE transposes write 4 tiles into one PSUM bank before a single
    512-wide copy (copy cost is latency-dominated).
"""

import numpy as np
import ml_dtypes

import concourse.bass as bass
import concourse.tile as tile
from concourse import bacc, mybir
from concourse import bass_utils
from concourse.masks import make_identity
from concourse.bass_interp import get_hw_module

F32 = mybir.dt.float32
BF16 = mybir.dt.bfloat16
FP8 = mybir.dt.float8e4
ADD = mybir.AluOpType.add
MULT = mybir.AluOpType.mult
DR = mybir.MatmulPerfMode.DoubleRow

B, N, D = 4, 2048, 1024
NQ = N // 2          # queries per core
KT = D // 128        # 8 contraction tiles
MT = N // 128        # 16 key tiles
QT = NQ // 128       # 8 query tiles
NCH = N // 512       # 4 key chunks of 512
SCALE = 1.0 / np.sqrt(np.float32(D))

MM_MODE = "fp8"

LAST_RESULT = None   # BassKernelResults of the most recent run (for test.py)
_CACHED = {}


def _build_fp8(debug=False):
    nc = bacc.Bacc(
        "TRN2",
        target_bir_lowering=False,
        debug=False,
        enable_asserts=False,
        num_devices=8,
    )
    # x^T chunk-major: [128, chunk, kt, 512]; bf16 for own-half chunks 0,1
    # and fp8 for partner-half chunks 2,3
    xt_d = nc.dram_tensor("xt", [128, 2, KT, 512], BF16,
                          kind="ExternalInput").ap()
    xt8_d = nc.dram_tensor("xt8", [128, 2, KT, 512], FP8,
                           kind="ExternalInput").ap()
    wspin_d = nc.dram_tensor("wspinT", [128, KT, D], BF16,
                             kind="ExternalInput").ap()
    wspin8_d = nc.dram_tensor("wspinT8", [128, KT, D], FP8,
                              kind="ExternalInput").ap()
    wj_d = nc.dram_tensor("wjT", [128, KT, D], FP8, kind="ExternalInput").ap()
    wglob_d = nc.dram_tensor("wglobT", [128, KT, D], BF16,
                             kind="ExternalInput").ap()
    bspin_d = nc.dram_tensor("bspin", [128, KT], F32, kind="ExternalInput").ap()
    beta_d = nc.dram_tensor("beta", [1, 1], F32, kind="ExternalInput").ap()
    out_d = nc.dram_tensor("out", [NQ, D], F32, kind="ExternalOutput").ap()

    with tile.TileContext(nc) as tc:
        with (
            tc.tile_pool(name="const", bufs=1) as const,
            tc.tile_pool(name="longp", bufs=1) as longp,
            tc.tile_pool(name="stats", bufs=8) as stats,
            tc.tile_pool(name="rinvp", bufs=4) as rinvp,
        ):
            ident32 = const.tile([128, 128], F32)
            make_identity(nc, ident32)
            ident_a = const.tile([128, 128], BF16)
            nc.vector.tensor_copy(ident_a[:], ident32[:])
            ones1 = const.tile([1, 128], BF16)
            nc.vector.memset(ones1, 1.0)
            beta_sb = const.tile([128, 1], F32)
            nc.gpsimd.dma_start(out=beta_sb[:], in_=beta_d.to_broadcast((128, 1)))
            bspin_sb = const.tile([128, KT], F32)
            nc.gpsimd.dma_start(out=bspin_sb[:], in_=bspin_d[:])
            mf4 = const.tile([128, KT, NCH], F32)
            mf = const.tile([128, KT], F32)
            mfs = const.tile([128, KT], BF16)
            gT = const.tile([1, D], BF16)
            G_sb = const.tile([128, D], F32)   # g broadcast to all partitions

            ST = longp.tile([128, KT, N], BF16)    # s^T: [d-in-tile, d-tile, key]
            ST8 = longp.tile([128, KT, N], FP8)    # fp8 copy for EF/scores moving
            SN = longp.tile([128, MT, D], FP8)     # [key-in-tile, key-tile, d]
            SQG = longp.tile([128, QT, D], F32)    # s + g for own queries

            # EF8 outlives phase 1; enter before ph1 (LIFO close order)
            efp_cm = tc.tile_pool(name="efp", bufs=1)
            efp = efp_cm.__enter__()
            EF8 = efp.tile([128, KT, NQ], FP8)  # [d-in-tile, d-tile, query]

            # ---- Phase 1 (+EF +transposes +g) ----
            with tc.tile_pool(name="ph1", bufs=1) as ph1:
                wspin_sb = ph1.tile([128, KT, D], BF16)
                wspin8_sb = ph1.tile([128, KT, D], FP8)
                wj8 = ph1.tile([128, KT, D], FP8)
                wglob_sb = ph1.tile([128, KT, D], BF16)
                xtc = {}

                def load_chunk(nch, kta=0, ktb=KT):
                    if nch not in xtc:
                        dt = BF16 if nch < 2 else FP8
                        xtc[nch] = ph1.tile([128, KT, 512], dt,
                                            name=f"xtc{nch}", tag=f"xtc{nch % 2}",
                                            bufs=1)
                    src = xt_d if nch < 2 else xt8_d
                    nc.sync.dma_start(
                        out=xtc[nch][:, kta:ktb, :],
                        in_=src[:, nch % 2, kta:ktb, :])

                # interleave weight/x loads so the first 8 matmuls are gated
                # on only ~0.4MB of DMA, and each later pass's data arrives
                # just ahead of the PE
                nc.sync.dma_start(out=wspin_sb[:, 0:1, :], in_=wspin_d[:, 0:1, :])
                load_chunk(0, 0, 1)
                nc.sync.dma_start(out=wspin_sb[:, 1:2, :], in_=wspin_d[:, 1:2, :])
                load_chunk(0, 1, 2)
                nc.sync.dma_start(out=wspin_sb[:, 2:4, :], in_=wspin_d[:, 2:4, :])
                load_chunk(0, 2, 4)
                nc.sync.dma_start(out=wspin_sb[:, 4:8, :], in_=wspin_d[:, 4:8, :])
                load_chunk(0, 4, 8)
                load_chunk(1)

                # chunk 0 in kt-split passes so matmuls start early
                with tc.tile_pool(name="ps1a", bufs=1, space="PSUM") as ps1a:
                    ps_n0 = [ps1a.tile([128, 512], F32, name=f"psn0_{ot}",
                                       tag=f"psn0_{ot}")
                             for ot in range(KT)]
                    kt0 = 0
                    for pi, klen in enumerate((1, 1, 2, 4)):
                        for ot in range(KT):
                            for kt in range(kt0, kt0 + klen):
                                nc.tensor.matmul(
                                    ps_n0[ot][:],
                                    wspin_sb[:, kt, ot * 128:(ot + 1) * 128],
                                    xtc[0][:, kt, :],
                                    start=(kt == 0), stop=(kt == KT - 1),
                                )
                        kt0 += klen
                        if pi == 0:
                            # queue the remaining input DMA behind the hot ones
                            nc.sync.dma_start(out=wj8[:], in_=wj_d[:])
                            nc.sync.dma_start(out=wspin8_sb[:], in_=wspin8_d[:])
                            load_chunk(2)
                            load_chunk(3)
                            nc.sync.dma_start(out=wglob_sb[:], in_=wglob_d[:])
                    for ot in range(KT):
                        nc.vector.tensor_scalar(
                            out=ST[:, ot, 0:512],
                            in0=ps_n0[ot][:],
                            scalar1=bspin_sb[:, ot:ot + 1],
                            scalar2=None,
                            op0=ADD, op1=ADD,
                            accum_out=mf4[:, ot, 0:1],
                        )
                        nc.scalar.copy(ST8[:, ot, 0:512], ST[:, ot, 0:512])

                with (
                    tc.tile_pool(name="ps1", bufs=3, space="PSUM") as ps1,
                    tc.tile_pool(name="ps2", bufs=1, space="PSUM") as ps2,
                    tc.tile_pool(name="ps3", bufs=3, space="PSUM") as ps3,
                ):
                    def s_epilogue(ps, ot, nch):
                        sl = slice(nch * 512, (nch + 1) * 512)
                        nc.vector.tensor_scalar(
                            out=ST[:, ot, sl],
                            in0=ps[:],
                            scalar1=bspin_sb[:, ot:ot + 1],
                            scalar2=None,
                            op0=ADD, op1=ADD,
                            accum_out=mf4[:, ot, nch:nch + 1],
                        )
                        nc.scalar.copy(ST8[:, ot, sl], ST[:, ot, sl])

                    # chunk 1: bf16 (own-half keys = own queries)
                    for ot in range(KT):
                        ps = ps1.tile([128, 512], F32)
                        for kt in range(KT):
                            nc.tensor.matmul(
                                ps[:],
                                wspin_sb[:, kt, ot * 128:(ot + 1) * 128],
                                xtc[1][:, kt, :],
                                start=(kt == 0), stop=(kt == KT - 1),
                            )
                        s_epilogue(ps, ot, 1)

                    # chunk 2: fp8 DoubleRow (partner half, attention-only)
                    for ot in range(KT):
                        ps = ps1.tile([128, 512], F32)
                        for j in range(KT // 2):
                            nc.tensor.matmul(
                                ps[:],
                                wspin8_sb[:, 2 * j:2 * j + 2,
                                          ot * 128:(ot + 1) * 128],
                                xtc[2][:, 2 * j:2 * j + 2, :],
                                start=(j == 0), stop=(j == KT // 2 - 1),
                                perf_mode=DR,
                            )
                        s_epilogue(ps, ot, 2)

                    # EF8 = W_J^T . s^T[queries] (needs only chunks 0,1)
                    for ot in range(KT):
                        for ch in range(2):
                            ps = ps1.tile([128, 512], F32)
                            for j in range(KT // 2):
                                nc.tensor.matmul(
                                    ps[:],
                                    wj8[:, 2 * j:2 * j + 2,
                                        ot * 128:(ot + 1) * 128],
                                    ST8[:, 2 * j:2 * j + 2,
                                        ch * 512:(ch + 1) * 512],
                                    start=(j == 0), stop=(j == KT // 2 - 1),
                                    perf_mode=DR,
                                )
                            nc.vector.tensor_copy(
                                EF8[:, ot, ch * 512:(ch + 1) * 512], ps[:]
                            )

                    # chunk 3: fp8 DoubleRow
                    for ot in range(KT):
                        ps = ps1.tile([128, 512], F32)
                        for j in range(KT // 2):
                            nc.tensor.matmul(
                                ps[:],
                                wspin8_sb[:, 2 * j:2 * j + 2,
                                          ot * 128:(ot + 1) * 128],
                                xtc[3][:, 2 * j:2 * j + 2, :],
                                start=(j == 0), stop=(j == KT // 2 - 1),
                                perf_mode=DR,
                            )
                        s_epilogue(ps, ot, 3)

                    # mean field (DVE; drains behind chunk-3 epilogues)
                    for ot in range(KT):
                        nc.vector.reduce_sum(
                            out=mf[:, ot:ot + 1], in_=mf4[:, ot, :],
                            axis=mybir.AxisListType.X,
                        )
                    nc.vector.tensor_scalar_mul(mfs[:], mf[:], 1.0 / N)

                    def transpose_block(mt):
                        for dq in range(KT // 4):
                            tp = ps3.tile([128, 4, 128], BF16, name="tp",
                                          tag="tp", bufs=3)
                            for j in range(4):
                                nc.tensor.transpose(
                                    tp[:, j, :],
                                    ST[:, dq * 4 + j, mt * 128:(mt + 1) * 128],
                                    ident_a[:],
                                )
                            dsl4 = slice(dq * 512, (dq + 1) * 512)
                            # split fp8 copies across ACT/DVE
                            if dq % 2 == 0:
                                nc.scalar.copy(SN[:, mt, dsl4], tp[:])
                            else:
                                nc.vector.tensor_copy(SN[:, mt, dsl4], tp[:])
                            if mt < QT:
                                # fold the constant g in: SQG = s + g
                                nc.vector.tensor_tensor(
                                    out=SQG[:, mt, dsl4],
                                    in0=tp[:].rearrange("p a b -> p (a b)"),
                                    in1=G_sb[:, dsl4],
                                    op=ADD,
                                )

                    # partner-half transposes (no SQG -> no dependence on g);
                    # they cover the mean-field DVE drain
                    for mt in range(QT, MT):
                        transpose_block(mt)

                    # g^T = mf^T . W_global^T, then broadcast to partitions
                    gps = ps2.tile([1, 2, 512], F32)
                    for ch in range(2):
                        for dt_ in range(KT):
                            nc.tensor.matmul(
                                gps[:, ch, :],
                                mfs[:, dt_:dt_ + 1],
                                wglob_sb[:, dt_, ch * 512:(ch + 1) * 512],
                                start=(dt_ == 0), stop=(dt_ == KT - 1),
                            )
                    nc.vector.tensor_copy(
                        gT[:], gps[0:1, :, :].rearrange("p a b -> p (a b)")
                    )
                    for ch in range(2):
                        ps = ps1.tile([128, 512], F32)
                        nc.tensor.matmul(
                            ps[:], ones1[:], gT[:, ch * 512:(ch + 1) * 512],
                            start=True, stop=True)
                        nc.scalar.copy(G_sb[:, ch * 512:(ch + 1) * 512], ps[:])

                    # own-half transposes (SQG needs G ready)
                    for mt in range(QT):
                        transpose_block(mt)

            # ---- Attention: scores(0,1), then the pipelined
            # scores/softmax + P-transpose/local-field loop.
            with (
                tc.tile_pool(name="work", bufs=2) as work,
                tc.tile_pool(name="ps5s", bufs=4, space="PSUM") as ps5s,
            ):
                def scores_softmax(qt):
                    q0 = qt * 128
                    P_sb = work.tile([128, N], BF16, bufs=4)
                    rs4 = stats.tile([128, NCH], F32)
                    for mch in range(NCH):
                        ps_s = ps5s.tile([128, 512], F32)
                        for j in range(KT // 2):
                            nc.tensor.matmul(
                                ps_s[:],
                                EF8[:, 2 * j:2 * j + 2, q0:q0 + 128],
                                ST8[:, 2 * j:2 * j + 2,
                                    mch * 512:(mch + 1) * 512],
                                start=(j == 0), stop=(j == KT // 2 - 1),
                                perf_mode=DR,
                            )
                        # no max subtraction: |scores|*SCALE < ~2 here.
                        # P stays UNNORMALIZED (fp8-friendly range).
                        nc.scalar.activation(
                            out=P_sb[:, mch * 512:(mch + 1) * 512],
                            in_=ps_s[:],
                            func=mybir.ActivationFunctionType.Exp,
                            bias=0.0, scale=float(SCALE),
                            accum_out=rs4[:, mch:mch + 1],
                        )
                    rs = stats.tile([128, 1], F32)
                    nc.vector.reduce_sum(out=rs[:], in_=rs4[:],
                                         axis=mybir.AxisListType.X)
                    rinv = rinvp.tile([128, 1], F32)
                    nc.vector.reciprocal(rinv[:], rs[:])
                    return P_sb, rinv

                live = {}
                live[0] = scores_softmax(0)
                live[1] = scores_softmax(1)

                ps5t_cm = tc.tile_pool(name="ps5t", bufs=2, space="PSUM")
                ps5t = ps5t_cm.__enter__()
                ps5l_cm = tc.tile_pool(name="ps5l", bufs=2, space="PSUM")
                ps5l = ps5l_cm.__enter__()

                def pt_lf(qt, P_rinv):
                    P_sb, rinv = P_rinv
                    q0 = qt * 128
                    PT = work.tile([128, MT, 128], FP8)
                    for mq in range(MT // 4):
                        tp2 = ps5t.tile([128, 4, 128], BF16)
                        for j in range(4):
                            mt = mq * 4 + j
                            nc.tensor.transpose(
                                tp2[:, j, :],
                                P_sb[:, mt * 128:(mt + 1) * 128],
                                ident_a[:],
                            )
                        nc.vector.tensor_copy(
                            PT[:, mq * 4:(mq + 1) * 4, :], tp2[:])
                    for dch in range(2):
                        dsl = slice(dch * 512, (dch + 1) * 512)
                        plf = ps5l.tile([128, 512], F32)
                        for j in range(MT // 2):
                            nc.tensor.matmul(
                                plf[:],
                                PT[:, 2 * j:2 * j + 2, :],
                                SN[:, 2 * j:2 * j + 2, dsl],
                                start=(j == 0), stop=(j == MT // 2 - 1),
                                perf_mode=DR,
                            )
                        # z = LF/Z + (s + g); out = tanh(beta * z)
                        z = work.tile([128, 512], F32)
                        nc.vector.scalar_tensor_tensor(
                            out=z[:], in0=plf[:], scalar=rinv[:],
                            in1=SQG[:, qt, dsl], op0=MULT, op1=ADD,
                        )
                        osb = work.tile([128, 512], F32, name="osb",
                                        tag="osb", bufs=4)
                        nc.scalar.activation(
                            out=osb[:], in_=z[:],
                            func=mybir.ActivationFunctionType.Tanh,
                            bias=0.0, scale=beta_sb[:],
                        )
                        nc.sync.dma_start(
                            out=out_d[q0:q0 + 128, dsl], in_=osb[:])

                for i in range(2, QT + 2):
                    if i < QT:
                        live[i] = scores_softmax(i)
                    pt_lf(i - 2, live.pop(i - 2))
                ps5l_cm.__exit__(None, None, None)
                ps5t_cm.__exit__(None, None, None)

            efp_cm.__exit__(None, None, None)

    nc.compile()
    nc.m = get_hw_module(nc.m)
    return nc


def _tile_kxm(a, np_dt):
    """(K, M) row-major -> [128, K//128, M] with k = kt*128 + p."""
    k, m = a.shape
    return np.ascontiguousarray(
        a.reshape(k // 128, 128, m).transpose(1, 0, 2)
    ).astype(np_dt)


def kernel(x, W_spin, b_spin, W_global, W_J, beta):
    global LAST_RESULT
    x = np.asarray(x, dtype=np.float32)
    W_spin = np.asarray(W_spin, dtype=np.float32)
    b_spin = np.asarray(b_spin, dtype=np.float32)
    W_global = np.asarray(W_global, dtype=np.float32)
    W_J = np.asarray(W_J, dtype=np.float32)
    beta = np.asarray(beta, dtype=np.float32)

    if MM_MODE not in _CACHED:
        _CACHED[MM_MODE] = _build_fp8()
    nc = _CACHED[MM_MODE]

    wspinT = _tile_kxm(W_spin.T, ml_dtypes.bfloat16)   # W_spin.T is (k, o)
    wspinT8 = _tile_kxm(W_spin.T, ml_dtypes.float8_e4m3)
    wjT = _tile_kxm(W_J.T, ml_dtypes.float8_e4m3)
    wglobT = _tile_kxm(W_global.T, ml_dtypes.bfloat16)
    bspin = np.ascontiguousarray(b_spin.reshape(KT, 128).T).astype(np.float32)
    beta_h = beta.reshape(1, 1).astype(np.float32)

    in_maps = []
    for core in range(8):
        b, h = divmod(core, 2)
        xb = x[b]
        if h == 0:
            x_perm = xb
        else:
            x_perm = np.concatenate([xb[NQ:], xb[:NQ]], axis=0)
        xt_full = _tile_kxm(np.ascontiguousarray(x_perm.T), np.float32)
        # [128, KT, N] -> chunk-major [128, NCH, KT, 512]
        xt_full = xt_full.reshape(128, KT, NCH, 512).transpose(0, 2, 1, 3)
        xt = np.ascontiguousarray(xt_full[:, 0:2]).astype(ml_dtypes.bfloat16)
        xt8 = np.ascontiguousarray(xt_full[:, 2:4]).astype(
            ml_dtypes.float8_e4m3)
        in_maps.append({
            "xt": xt, "xt8": xt8, "wspinT": wspinT, "wspinT8": wspinT8,
            "wjT": wjT, "wglobT": wglobT, "bspin": bspin, "beta": beta_h,
        })

    LAST_RESULT = bass_utils.run_bass_kernel_spmd(
        nc, in_maps, core_ids=list(range(8))
    )

    out = np.empty((B, N, D), dtype=np.float32)
    for core in range(8):
        b, h = divmod(core, 2)
        out[b, h * NQ:(h + 1) * NQ, :] = LAST_RESULT.results[core]["out"]
    return out
